# revision 1
# baseline (speedup 1.0000x reference)
"""Bass/Trainium2 kernel for nn_BayesConv2dMF (per-sample-weight 3x3 conv).

Contract: kernel(**inputs) takes FULL unsharded inputs
  input      [32, 128, 56, 56] f32
  eps        [32, 128, 128, 3, 3] f32
  weight_psi [128, 128, 3, 3] f32
  weight_mu  [128, 128, 3, 3] f32
and returns the FULL output [32, 128, 56, 56] f32.

Strategy: data-parallel over batch across 8 NeuronCores (4 images/core).
Per image on-core (software-pipelined one image ahead):
  wm = eps * exp(psi) in bf16                   (DVE; exp(psi) on ScalarE, once)
  wT[ci,k,co]: per-tap PE transpose of wm, with the shared muT (transposed
      once at startup -- transpose is linear) added during the PSUM
      evacuation on DVE
  x  -> zero-padded [CI, 58, 58] bf16 via SWDGE cast-DMA (GpSimd ring)
  conv: 2-chunk parts of 7 output rows each (1-chunk taper on the final
      image so the last store overlaps compute); taps outer so one weight
      load feeds the live chunks; 9 PSUM-accumulating matmuls per chunk
      (K=CI=128, N=7*56=392, bf16)
  PSUM -> SBUF (ScalarE, DVE on the last part) -> DRAM (SP HWDGE ring)
  plus dummy-matmul HAM warm-up bursts during the input ramp so the PE
  clock gate is released before the real conv stream starts

Cost-model time per core: ~63.9 us (conv stream 98% dense at the bf16
roofline ~47 us; DMA ~45.5 us of 16.3 MB at 360 GB/s; ramp ~9.4 us is
the serial first-image DMA floor; tail ~4 us stores+drain).
"""

import numpy as np

import concourse.bass as bass
import concourse.tile as tile
from concourse import bacc, mybir
from concourse.bass_utils import run_bass_kernel_spmd
from concourse.masks import make_identity

B, CO, CI, KH, KW, H, W = 32, 128, 128, 3, 3, 56, 56
K9 = KH * KW
N_CORES = 8
BPC = B // N_CORES  # images per core
HP, WP = H + 2, W + 2  # padded image
RB = 8  # output rows per PSUM chunk
NCHUNK = H // RB
F32 = mybir.dt.float32
BF16 = mybir.dt.bfloat16


def emit(nc, tc, ctx, x_d, eps_d, psi_d, mu_d, out_d):
    const = ctx.enter_context(tc.tile_pool(name="const", bufs=1))
    wpool = ctx.enter_context(tc.tile_pool(name="wpool", bufs=2))
    opool = ctx.enter_context(tc.tile_pool(name="opool", bufs=2))
    psw = ctx.enter_context(tc.tile_pool(name="psw", bufs=1, space="PSUM"))
    pso = ctx.enter_context(tc.tile_pool(name="pso", bufs=1, space="PSUM"))

    ident = const.tile([128, 128], BF16)
    make_identity(nc, ident)
    ident_f = const.tile([128, 128], F32)
    make_identity(nc, ident_f)

    psi_t = const.tile([CO, CI, K9], F32)
    nc.sync.dma_start(psi_t, psi_d.rearrange("co ci kh kw -> co ci (kh kw)"))
    exp_psi = const.tile([CO, CI, K9], F32)
    nc.scalar.activation(exp_psi, psi_t, mybir.ActivationFunctionType.Exp)
    mu_t = const.tile([CO, CI, K9], F32)
    nc.sync.dma_start(mu_t, mu_d.rearrange("co ci kh kw -> co ci (kh kw)"))

    muT = const.tile([CI, K9, CO], F32)

    def emit_muT():
        # one-time: muT[ci, k, co] = mu[co, ci, k] via fp32 PE transposes
        # (transpose is linear, so wT = (eps*exp(psi))^T + muT)
        for g in range(3):
            psum_mut = psw.tile(
                [CI, 3, CO], F32, tag="psmut", name=f"psmut{g}", bufs=1
            )
            for j in range(3):
                k = 3 * g + j
                nc.tensor.transpose(psum_mut[:, j, :], mu_t[:, :, k], ident_f)
            nc.vector.tensor_copy(muT[:, 3 * g : 3 * g + 3, :], psum_mut)

    # HAM warm-up: the PE sits idle for ~6 us during the input ramp and
    # would enter the first conv matmuls clock-gated at 1.2 GHz. Burn the
    # idle window with dummy matmuls (identity x identity) so the activity
    # monitor releases the gate before the real stream starts. Results go
    # to a scratch PSUM slot nobody reads.
    warm_ps = psw.tile([128, 128], F32, tag="psmut", name="warm_ps", bufs=1)
    for i in range(56):
        nc.tensor.matmul(warm_ps, ident, ident, start=True, stop=True)
    # second burst rides on mu's arrival so the activity bridges the gap
    # until the real conv stream begins (MID-window re-throttle is ~3.4 us)
    warm_ps2 = psw.tile([128, 128], F32, tag="psmut", name="warm_ps2", bufs=1)
    for i in range(10):
        nc.tensor.matmul(warm_ps2, mu_t[:, :, 0], ident_f, start=True, stop=True)

    # persistent padded-input tiles; borders stay zero across images
    NXP = 3
    xpads = []
    for i in range(NXP):
        xp = const.tile([CI, HP, WP], BF16, name=f"xpad{i}", tag=f"xpad{i}")
        # only the borders need zeroing (interior is overwritten every image)
        nc.vector.memset(xp[:, 0, :], 0.0)
        nc.vector.memset(xp[:, HP - 1, :], 0.0)
        nc.vector.memset(xp[:, 1 : HP - 1, 0 : WP : WP - 1], 0.0)
        xpads.append(xp)

    HALF = H // 2  # 28 output rows per half
    RB2 = 7  # rows per PSUM chunk
    NCH = HALF // RB2  # 4 chunks per half, all live in PSUM (k-outer loop)

    wTs = {}
    out_sbs = {}
    last_x_dma = {}

    def prep(b):
        # per-sample weights: wm = eps * exp(psi) in bf16 (natural layout),
        # transpose each tap, add muT during the PSUM evacuation.
        # Image 0 takes the direct path (add natural mu before transposing)
        # so its critical chain doesn't wait for muT.
        eps_t = wpool.tile([CO, CI, K9], F32, tag="eps", name=f"eps{b}")
        nc.sync.dma_start(
            eps_t, eps_d[b].rearrange("co ci kh kw -> co ci (kh kw)")
        )
        w_bf = wpool.tile([CO, CI, K9], BF16, tag="wbf", name=f"wbf{b}")
        psum_wt = psw.tile([CI, K9, CO], BF16, tag="pswt", name=f"pswt{b}")
        wT = wpool.tile([CI, K9, CO], BF16, tag="wT", name=f"wT{b}")
        if b == 0:
            # image 0 is ramp-critical: pipeline the weight chain per
            # 3-tap group (direct mu add; no muT dependency)
            for g in range(3):
                sl = slice(3 * g, 3 * g + 3)
                nc.vector.tensor_mul(
                    eps_t[:, :, sl], eps_t[:, :, sl], exp_psi[:, :, sl]
                )
                nc.vector.tensor_add(
                    w_bf[:, :, sl], eps_t[:, :, sl], mu_t[:, :, sl]
                )
                for k in range(3 * g, 3 * g + 3):
                    nc.tensor.transpose(psum_wt[:, k, :], w_bf[:, :, k], ident)
                nc.scalar.copy(wT[:, sl, :], psum_wt[:, sl, :])
        else:
            nc.vector.tensor_mul(w_bf, eps_t, exp_psi)
            for k in range(K9):
                nc.tensor.transpose(psum_wt[:, k, :], w_bf[:, :, k], ident)
            nc.vector.tensor_add(wT, psum_wt, muT)
        wTs[b] = wT

        # input image: SWDGE cast-DMA (f32->bf16) straight into the padded
        # tile, split so early conv parts can start before the full image
        # lands (image 0 is ramp-critical -> 4 pieces)
        xp = xpads[b % NXP]
        bounds = [0, 15, 29, 43, H] if b == 0 else [0, HALF + 2, H]
        for lo, hi in zip(bounds[:-1], bounds[1:]):
            last_x_dma[b] = nc.gpsimd.dma_start(
                xp[:, lo + 1 : hi + 1, 1 : W + 1], x_d[b][:, lo:hi, :]
            )
        out_sbs[b] = opool.tile([CO, H, W], F32, tag="osb", name=f"osb{b}")

    def conv_part(b, r0, nch, pso_off, last=False):
        xp = xpads[b % NXP]
        wT = wTs[b]
        out_sb = out_sbs[b]
        rows = nch * RB2
        pss = []
        for c in range(nch):
            ps = pso.tile(
                [CO, RB2, W],
                F32,
                tag=f"pso{pso_off + c}",
                name=f"ps{pso_off + c}",
                bufs=2 if pso_off + c < 1 else 1,
            )
            pss.append(ps)
        # taps outer: one weight load per tap feeds all live chunk matmuls
        for k in range(K9):
            kh, kw = divmod(k, KW)
            for c in range(nch):
                rr = r0 + c * RB2 + kh
                nc.tensor.matmul(
                    pss[c],
                    wT[:, k, :],
                    xp[:, rr : rr + RB2, kw : kw + W],
                    start=(k == 0),
                    stop=(k == K9 - 1),
                )
        for c in range(nch):
            dst = out_sb[:, r0 + c * RB2 : r0 + (c + 1) * RB2, :]
            # steady state: keep DVE free for the next image's weight path
            if last and c % 2 == 1:
                nc.vector.tensor_copy(dst, pss[c])
            else:
                nc.scalar.copy(dst, pss[c])
        nc.sync.dma_start(
            out_d[b][:, r0 : r0 + rows, :], out_sb[:, r0 : r0 + rows, :]
        )

    # software-pipelined emission: image b+1's weight/x prep is emitted
    # between the conv parts of image b so its DVE/PE work interleaves.
    # The final image ends with two small 2-chunk parts so the last store
    # overlaps compute and the drain tail shrinks.
    prep(0)
    emit_muT()
    for b in range(BPC):
        conv_part(b, 0, 2, 0)
        conv_part(b, 2 * RB2, 2, 2)
        if b + 1 < BPC:
            prep(b + 1)
            conv_part(b, HALF, 2, 0)
            conv_part(b, HALF + 2 * RB2, 2, 2)
        else:
            conv_part(b, HALF, 1, 0)
            conv_part(b, HALF + RB2, 1, 1)
            conv_part(b, HALF + 2 * RB2, 1, 2)
            conv_part(b, HALF + 3 * RB2, 1, 3, last=True)


def build():
    from contextlib import ExitStack

    nc = bacc.Bacc("TRN2", target_bir_lowering=False, debug=False, num_devices=N_CORES)
    x_d = nc.dram_tensor("input", [BPC, CI, H, W], F32, kind="ExternalInput").ap()
    eps_d = nc.dram_tensor(
        "eps", [BPC, CO, CI, KH, KW], F32, kind="ExternalInput"
    ).ap()
    psi_d = nc.dram_tensor(
        "weight_psi", [CO, CI, KH, KW], F32, kind="ExternalInput"
    ).ap()
    mu_d = nc.dram_tensor("weight_mu", [CO, CI, KH, KW], F32, kind="ExternalInput").ap()
    out_d = nc.dram_tensor("out", [BPC, CO, H, W], F32, kind="ExternalOutput").ap()

    with tile.TileContext(nc) as tc:
        with ExitStack() as ctx:
            emit(nc, tc, ctx, x_d, eps_d, psi_d, mu_d, out_d)
    nc.compile()
    return nc


_NC_CACHE = None


def kernel(input, eps, weight_psi, weight_mu, **run_kwargs):
    global _NC_CACHE
    if _NC_CACHE is None:
        _NC_CACHE = build()
    nc = _NC_CACHE
    in_maps = []
    for c in range(N_CORES):
        sl = slice(c * BPC, (c + 1) * BPC)
        in_maps.append(
            {
                "input": np.ascontiguousarray(input[sl], dtype=np.float32),
                "eps": np.ascontiguousarray(eps[sl], dtype=np.float32),
                "weight_psi": np.ascontiguousarray(weight_psi, dtype=np.float32),
                "weight_mu": np.ascontiguousarray(weight_mu, dtype=np.float32),
            }
        )
    res = run_bass_kernel_spmd(
        nc, in_maps, core_ids=list(range(N_CORES)), **run_kwargs
    )
    out = np.concatenate([res.results[c]["out"] for c in range(N_CORES)], axis=0)
    kernel._last_results = res
    return out



# revision 3
# speedup vs baseline: 1.0473x; 1.0473x over previous
"""Bass/Trainium2 kernel for nn_BayesConv2dMF (per-sample-weight 3x3 conv).

Contract: kernel(**inputs) takes FULL unsharded inputs
  input      [32, 128, 56, 56] f32
  eps        [32, 128, 128, 3, 3] f32
  weight_psi [128, 128, 3, 3] f32
  weight_mu  [128, 128, 3, 3] f32
and returns the FULL output [32, 128, 56, 56] f32.

Strategy: data-parallel over batch across 8 NeuronCores (4 images/core).
All DMA is bf16: weight tensors and the input are cast-loaded f32->bf16
with fully-contiguous descriptors (>=512B runs -> full DMA rate), and the
output is stored as bf16 then upcast to f32 on the host. Per image:
  w  = eps * exp(psi) + mu in bf16 (DVE, 2x mode)
  wT[ci,k,co] per-tap PE transposes, evacuated by ScalarE
  conv: 4 parts of 2 chunks x 7 output rows; 9 PSUM-accumulating
  matmuls per chunk (K=CI=128, bf16). No padded input tile: border taps
  use trimmed access patterns (center tap first carries start=True).
  PSUM -> SBUF bf16 (ScalarE) -> DRAM bf16 (SP HWDGE)
Dummy-matmul warm-up bursts release the PE clock gate during the ramp.
"""

import numpy as np

import concourse.bass as bass
import concourse.tile as tile
from concourse import bacc, mybir
from concourse.bass_utils import run_bass_kernel_spmd
from concourse.masks import make_identity

B, CO, CI, KH, KW, H, W = 32, 128, 128, 3, 3, 56, 56
K9 = KH * KW
N_CORES = 8
BPC = B // N_CORES  # images per core
RB = 7  # output rows per PSUM chunk
F32 = mybir.dt.float32
BF16 = mybir.dt.bfloat16

N_WARMUP = 62


def emit(nc, tc, ctx, x_d, eps_d, psi_d, mu_d, out_d):
    const = ctx.enter_context(tc.tile_pool(name="const", bufs=1))
    wpool = ctx.enter_context(tc.tile_pool(name="wpool", bufs=2))
    opool = ctx.enter_context(tc.tile_pool(name="opool", bufs=2))
    psw = ctx.enter_context(tc.tile_pool(name="psw", bufs=1, space="PSUM"))
    pso = ctx.enter_context(tc.tile_pool(name="pso", bufs=1, space="PSUM"))

    ident = const.tile([128, 128], BF16)
    make_identity(nc, ident)

    # weight constants, cast-loaded to bf16 (SWDGE), contiguous full rate
    psi_t = const.tile([CO, CI, K9], BF16)
    nc.gpsimd.dma_start(psi_t, psi_d.rearrange("co ci kh kw -> co ci (kh kw)"))
    exp_psi = const.tile([CO, CI, K9], BF16)
    nc.scalar.activation(exp_psi, psi_t, mybir.ActivationFunctionType.Exp)
    mu_t = const.tile([CO, CI, K9], BF16)
    nc.gpsimd.dma_start(mu_t, mu_d.rearrange("co ci kh kw -> co ci (kh kw)"))

    # HAM warm-up: keep the PE busy during the input ramp so the activity
    # monitor releases the clock gate before the real conv stream starts.
    warm_ps = psw.tile([128, 128], F32, tag="warm", name="warm_ps", bufs=1)
    for i in range(N_WARMUP):
        nc.tensor.matmul(warm_ps, ident, ident, start=True, stop=True)

    xls = []
    for i in range(3):
        xls.append(const.tile([CI, H, W], BF16, name=f"xl{i}", tag=f"xl{i}"))

    wTs = {}
    out_sbs = {}

    def prep(b):
        # per-sample weights: w = eps * exp(psi) + mu, all bf16
        eps_t = wpool.tile([CO, CI, K9], BF16, tag="eps", name=f"eps{b}")
        nc.gpsimd.dma_start(
            eps_t, eps_d[b].rearrange("co ci kh kw -> co ci (kh kw)")
        )
        w_bf = wpool.tile([CO, CI, K9], BF16, tag="wbf", name=f"wbf{b}")
        psum_wt = psw.tile([CI, K9, CO], BF16, tag="pswt", name=f"pswt{b}")
        wT = wpool.tile([CI, K9, CO], BF16, tag="wT", name=f"wT{b}")
        if b == 0:
            # ramp-critical: pipeline the weight chain per 5/4-tap group
            for lo, hi in ((0, 5), (5, 9)):
                sl = slice(lo, hi)
                nc.vector.tensor_mul(
                    eps_t[:, :, sl], eps_t[:, :, sl], exp_psi[:, :, sl]
                )
                nc.vector.tensor_add(
                    w_bf[:, :, sl], eps_t[:, :, sl], mu_t[:, :, sl]
                )
                for k in range(lo, hi):
                    nc.tensor.transpose(psum_wt[:, k, :], w_bf[:, :, k], ident)
                nc.scalar.copy(wT[:, sl, :], psum_wt[:, sl, :])
        else:
            nc.vector.tensor_mul(eps_t, eps_t, exp_psi)
            nc.vector.tensor_add(w_bf, eps_t, mu_t)
            for k in range(K9):
                nc.tensor.transpose(psum_wt[:, k, :], w_bf[:, :, k], ident)
            nc.scalar.copy(wT, psum_wt)
        wTs[b] = wT

        # input image: contiguous cast-DMA f32->bf16, two halves so the
        # first conv part can start before the full image lands
        xl = xls[b % 3]
        bounds = [0, 14, 28, H] if b == 0 else [0, H // 2, H]
        for lo, hi in zip(bounds[:-1], bounds[1:]):
            nc.gpsimd.dma_start(xl[:, lo:hi, :], x_d[b][:, lo:hi, :])
        out_sbs[b] = opool.tile([CO, H, W], BF16, tag="osb", name=f"osb{b}")

    # tap order: center tap first (full coverage => carries start=True)
    TAPS = [4, 0, 1, 2, 3, 5, 6, 7, 8]

    def conv_part(b, r0, nch):
        """Emit one part: `nch` chunks of RB rows starting at output row r0."""
        xl = xls[b % 3]
        wT = wTs[b]
        out_sb = out_sbs[b]
        pss = []
        for c in range(nch):
            ps = pso.tile(
                [CO, RB, W], F32,
                tag=f"pso{(r0 // RB + c) % 4}",
                name=f"ps_{b}_{r0 + c * RB}",
            )
            pss.append(ps)
        for ki, k in enumerate(TAPS):
            kh, kw = divmod(k, KW)
            # col trim (same for every chunk)
            clo, chi = max(0, 1 - kw), min(W, W + 1 - kw)
            for c in range(nch):
                rr = r0 + c * RB
                olo = max(0, 1 - kh - rr)
                ohi = min(RB, H + 1 - kh - rr)
                ilo = rr + olo + kh - 1
                nc.tensor.matmul(
                    pss[c][:, olo:ohi, clo:chi],
                    wT[:, k, :],
                    xl[:, ilo : ilo + ohi - olo, clo + kw - 1 : chi + kw - 1],
                    start=(ki == 0),
                    stop=(ki == len(TAPS) - 1),
                )
        for c in range(nch):
            nc.scalar.copy(out_sb[:, r0 + c * RB : r0 + (c + 1) * RB, :], pss[c])
        rows = nch * RB
        nc.sync.dma_start(
            out_d[b][:, r0 : r0 + rows, :], out_sb[:, r0 : r0 + rows, :]
        )

    # software-pipelined emission: image b+1's prep is emitted between the
    # conv parts of image b. The final image tapers to 1-chunk parts so the
    # last store is small and the drain tail shrinks.
    prep(0)
    for b in range(BPC):
        conv_part(b, 0, 2)
        conv_part(b, 2 * RB, 2)
        if b + 1 < BPC:
            prep(b + 1)
            conv_part(b, 4 * RB, 2)
            conv_part(b, 6 * RB, 2)
        else:
            conv_part(b, 4 * RB, 2)
            conv_part(b, 6 * RB, 1)
            conv_part(b, 7 * RB, 1)


def build():
    from contextlib import ExitStack

    nc = bacc.Bacc("TRN2", target_bir_lowering=False, debug=False, num_devices=N_CORES)
    x_d = nc.dram_tensor("input", [BPC, CI, H, W], F32, kind="ExternalInput").ap()
    eps_d = nc.dram_tensor(
        "eps", [BPC, CO, CI, KH, KW], F32, kind="ExternalInput"
    ).ap()
    psi_d = nc.dram_tensor(
        "weight_psi", [CO, CI, KH, KW], F32, kind="ExternalInput"
    ).ap()
    mu_d = nc.dram_tensor("weight_mu", [CO, CI, KH, KW], F32, kind="ExternalInput").ap()
    out_d = nc.dram_tensor("out", [BPC, CO, H, W], BF16, kind="ExternalOutput").ap()

    with tile.TileContext(nc) as tc:
        with ExitStack() as ctx:
            emit(nc, tc, ctx, x_d, eps_d, psi_d, mu_d, out_d)
    nc.compile()
    return nc


_NC_CACHE = None


def kernel(input, eps, weight_psi, weight_mu, **run_kwargs):
    global _NC_CACHE
    if _NC_CACHE is None:
        _NC_CACHE = build()
    nc = _NC_CACHE
    in_maps = []
    for c in range(N_CORES):
        sl = slice(c * BPC, (c + 1) * BPC)
        in_maps.append(
            {
                "input": np.ascontiguousarray(input[sl], dtype=np.float32),
                "eps": np.ascontiguousarray(eps[sl], dtype=np.float32),
                "weight_psi": np.ascontiguousarray(weight_psi, dtype=np.float32),
                "weight_mu": np.ascontiguousarray(weight_mu, dtype=np.float32),
            }
        )
    res = run_bass_kernel_spmd(
        nc, in_maps, core_ids=list(range(N_CORES)), **run_kwargs
    )
    out = np.concatenate(
        [np.asarray(res.results[c]["out"]) for c in range(N_CORES)], axis=0
    ).astype(np.float32)
    kernel._last_results = res
    return out
